# revision 26
# baseline (speedup 1.0000x reference)
"""Multi-head attention (B=8, N=1024, C=768, H=12) on 8 TRN2 NeuronCores.

Sharding: data-parallel - one batch element per core, weights replicated.
No collectives.

v2 design (vs baseline): bf16 matmul operands everywhere (f32 PSUM), query
dim split in 512-halves so PSUM fits 8 banks with QKV interleave slots,
software-pipelined S->exp->PV per j-tile, QKV/V generation interleaved into
the attention loop as PE gap-filler (keeps HAM warm), reciprocal via the
fast custom-DVE approx, partition broadcast on GPSIMD instead of 7 chained
DMAs.

Per-core dataflow:
  qt/kt [128, pair, 1024]: rows = head-pair dims (A at 0:64, B at 64:128).
  v_sb [128 j, jt, head, 65]: col 64 is ones -> PV row 64 = softmax denom.
  Per (pair t, half ib, jtile): S^T halves via two K=64 matmuls row-packed
  into the PE halves, one exp ACTIVATE (FD=1024, scale fused), PV accumulates
  O^T[65, 512] over jt. Normalize: denom row -> reciprocal_approx_fast ->
  partition_broadcast -> multiply into ot_sb (bf16).
  Proj: y = OT.T @ wp + bias per 128-row tile, straight to DRAM.
"""

from contextlib import ExitStack

import numpy as np

import concourse.bacc as bacc
import concourse.mybir as mybir
import concourse.tile as tile
from concourse.bass_utils import run_bass_kernel_spmd

F32 = mybir.dt.float32
BF16 = mybir.dt.bfloat16

B, N, C = 8, 1024, 768
H, HD = 12, 64
SCALE = HD ** -0.5
NT_I = N // 128   # 8 i/j tiles
NT_C = C // 128   # 6 c tiles (== head pairs)
NPAIR = H // 2    # 6


def build():
    nc = bacc.Bacc(None, target_bir_lowering=False)

    xt = nc.dram_tensor("xt", [C, N], BF16, kind="ExternalInput")
    wq = nc.dram_tensor("wq", [C, C], BF16, kind="ExternalInput")
    wk = nc.dram_tensor("wk", [C, C], BF16, kind="ExternalInput")
    wv = nc.dram_tensor("wv", [C, C], BF16, kind="ExternalInput")
    wp = nc.dram_tensor("wp", [C, C], BF16, kind="ExternalInput")
    bias = nc.dram_tensor("bias", [128, C], F32, kind="ExternalInput")
    y = nc.dram_tensor("y", [N, C], F32, kind="ExternalOutput")

    with tile.TileContext(nc) as tc, ExitStack() as stack:
        pp = stack.enter_context(tc.tile_pool(name="persist", bufs=1))
        p_pt = stack.enter_context(tc.tile_pool(name="pt", bufs=6))
        p_nrm = stack.enter_context(tc.tile_pool(name="nrm", bufs=2))
        p_y = stack.enter_context(tc.tile_pool(name="yout", bufs=2))
        ps_qkv = stack.enter_context(
            tc.tile_pool(name="psq", bufs=2, space="PSUM"))
        ps_st = stack.enter_context(
            tc.tile_pool(name="psst", bufs=2, space="PSUM"))
        ps_ov = stack.enter_context(
            tc.tile_pool(name="psov", bufs=2, space="PSUM"))

        xt_sb = pp.tile([128, NT_C, N], BF16)
        wq_sb = pp.tile([128, NT_C, C], BF16)
        wk_sb = pp.tile([128, NT_C, C], BF16)
        wv_sb = pp.tile([128, NT_C, C], BF16)
        wp_sb = pp.tile([128, NT_C, C], BF16)
        bias_sb = pp.tile([128, C], F32)
        qt_sb = pp.tile([128, NPAIR, N], BF16)
        kt_sb = pp.tile([128, NPAIR, N], BF16)
        v_sb = pp.tile([128, NT_I, H, HD + 1], BF16)
        ot_sb = pp.tile([128, NPAIR, N], BF16)

        # Input DMAs: few big strided transfers in exact need-order. The
        # first 4 attention steps need only wq/wk t=0 columns, the xt
        # ib0-half (feeds qt[0] ib0 + kt[0] j 0:512) — ~1.2MB, landing in
        # a few us. Everything needed later goes on GpSimd (NOT Scalar:
        # DMA sem-waits there would block the exp stream).
        wqv = wq.rearrange("(t p) d -> p t d", p=128)
        wkv = wk.rearrange("(t p) d -> p t d", p=128)
        xtv = xt.rearrange("(t p) i -> p t i", p=128)
        nc.sync.dma_start(wq_sb[:, :, 0:128], wqv[:, :, 0:128])
        nc.sync.dma_start(wk_sb[:, :, 0:128], wkv[:, :, 0:128])
        nc.sync.dma_start(xt_sb[:, :, 0:512], xtv[:, :, 0:512])
        nc.sync.dma_start(xt_sb[:, :, 512:1024], xtv[:, :, 512:1024])
        nc.sync.dma_start(wv_sb[:], wv.rearrange("(t p) d -> p t d", p=128))
        nc.gpsimd.dma_start(wq_sb[:, :, 128:C], wqv[:, :, 128:C])
        nc.gpsimd.dma_start(wk_sb[:, :, 128:C], wkv[:, :, 128:C])
        nc.gpsimd.dma_start(wp_sb[:], wp.rearrange("(t p) d -> p t d", p=128))
        nc.gpsimd.dma_start(bias_sb[:], bias[:])
        nc.vector.memset(v_sb[:, :, :, HD:HD + 1], 1.0)

        def gen_qk_chunk(t, which, ch):
            """One accumulation chain of Q.T (which=0) or K.T (which=1)."""
            w_sb, out_sb = ((wq_sb, qt_sb), (wk_sb, kt_sb))[which]
            acc = ps_qkv.tile([128, 512], F32, tag="acc",
                              name=f"qk{t}_{which}_{ch}")
            for k in range(NT_C):
                nc.tensor.matmul(
                    acc[:],
                    w_sb[:, k, t * 128:(t + 1) * 128],
                    xt_sb[:, k, ch * 512:(ch + 1) * 512],
                    start=(k == 0), stop=(k == NT_C - 1),
                )
            nc.vector.tensor_copy(out_sb[:, t, ch * 512:(ch + 1) * 512],
                                  acc[:])

        def gen_v_chunk(jt, ch):
            acc = ps_qkv.tile([128, 384], F32, tag="acc",
                              name=f"v{jt}_{ch}")
            for k in range(NT_C):
                nc.tensor.matmul(
                    acc[:],
                    xt_sb[:, k, jt * 128:(jt + 1) * 128],
                    wv_sb[:, k, ch * 384:(ch + 1) * 384],
                    start=(k == 0), stop=(k == NT_C - 1),
                )
            nc.vector.tensor_copy(
                v_sb[:, jt, 6 * ch:6 * ch + 6, 0:HD],
                acc[:].rearrange("p (h e) -> p h e", e=HD),
            )

        def attn_pair(t, ib, filler):
            """Attention for head pair t on query half ib (512 queries).

            filler: list of zero-arg emitters (extra PE work) drained a few
            per jt step so the scheduler has gap-fill matmuls while ACT
            runs exp. Emitted at the BOTTOM of each step so they rank below
            the attention instructions in scheduler priority (pure
            gap-fill). Drained fast enough that all run by step NT_I-2.
            """
            i0 = ib * 512
            hA, hB = 2 * t, 2 * t + 1
            per_step = -(-len(filler) // (NT_I - 1)) if filler else 0
            ovA = ps_ov.tile([HD + 1, 512], F32, tag="ov",
                             name=f"ovA{t}_{ib}")
            ovB = ps_ov.tile([HD + 1, 512], F32, tag="ov",
                             name=f"ovB{t}_{ib}")
            pts = [None] * NT_I
            for jt in range(NT_I + 1):
                if jt < NT_I:
                    st = ps_st.tile([128, 1024], F32, tag="st",
                                    name=f"st{t}_{ib}_{jt}")
                    nc.tensor.matmul(
                        st[:, 0:512],
                        kt_sb[0:64, t, jt * 128:(jt + 1) * 128],
                        qt_sb[0:64, t, i0:i0 + 512],
                    )
                    nc.tensor.matmul(
                        st[:, 512:1024],
                        kt_sb[64:128, t, jt * 128:(jt + 1) * 128],
                        qt_sb[64:128, t, i0:i0 + 512],
                    )
                    pt = p_pt.tile([128, 1024], BF16, tag="pt")
                    nc.scalar.activation(
                        pt[:], st[:],
                        mybir.ActivationFunctionType.Exp, scale=SCALE,
                    )
                    pts[jt] = pt
                if jt > 0:
                    j = jt - 1
                    pt = pts[j]
                    nc.tensor.matmul(
                        ovA[:], v_sb[:, j, hA, :], pt[:, 0:512],
                        start=(j == 0), stop=(j == NT_I - 1),
                    )
                    nc.tensor.matmul(
                        ovB[:], v_sb[:, j, hB, :], pt[:, 512:1024],
                        start=(j == 0), stop=(j == NT_I - 1),
                    )
                for _ in range(per_step):
                    if filler:
                        filler.pop(0)()
            # normalize: denom row 64 -> 1/l -> broadcast -> multiply
            for base, ov in ((0, ovA), (64, ovB)):
                rl = p_nrm.tile([1, 512], F32, tag="rl")
                rc = p_nrm.tile([1, 512], F32, tag="rc")
                bc = p_nrm.tile([128, 512], F32, tag="bc")
                nc.vector.tensor_copy(rl[0:1, :], ov[64:65, :])
                nc.vector.reciprocal_approx_fast(rc[0:1, :], rl[0:1, :])
                nc.gpsimd.partition_broadcast(bc[:], rc[0:1, :])
                nc.vector.tensor_mul(
                    ot_sb[base:base + 64, t, i0:i0 + 512],
                    ov[0:64, :],
                    bc[base:base + 64, :],
                )

        y_part = pp.tile([128, NT_I, C], F32)

        def proj_part(it, ks, first, last):
            """Partial projection over c_in chunks ks; accumulates into
            y_part (first partial also folds in the bias), DMAs out on the
            last partial."""
            y_sb = (p_y.tile([128, C], F32, tag="y", name=f"y{it}")
                    if last else None)
            for ch in range(2):
                acc = ps_qkv.tile([128, 384], F32, tag="acc",
                                  name=f"p{it}_{ks[0]}_{ch}")
                for k in ks:
                    nc.tensor.matmul(
                        acc[:],
                        ot_sb[:, k, it * 128:(it + 1) * 128],
                        wp_sb[:, k, ch * 384:(ch + 1) * 384],
                        start=(k == ks[0]), stop=(k == ks[-1]),
                    )
                sl = slice(ch * 384, (ch + 1) * 384)
                prev = bias_sb if first else y_part[:, it, :]
                dst = y_sb if last else y_part[:, it, :]
                nc.vector.tensor_add(dst[:, sl], acc[:], prev[:, sl])
            if last:
                eng = (nc.sync, nc.gpsimd)[it % 2]
                eng.dma_start(y[it * 128:(it + 1) * 128, :], y_sb[:])

        # prologue: Q.T/K.T chains for pair 0 (V rides in the fillers so
        # a late wv DMA cannot block the first S matmuls in PE order)
        for ch in range(2):
            gen_qk_chunk(0, 0, ch)
            gen_qk_chunk(0, 1, ch)

        for t in range(NPAIR):
            # fillers: extra PE chains emitted at the bottom of the
            # attention jt steps so the scheduler has gap-fill matmuls
            # while ACT runs exp.
            f0, f1 = [], []
            if t == 0:
                # V tiles inside pair 0; V[j] lands before the PV reads it
                f0 = [lambda jt=jt, ch=ch: gen_v_chunk(jt, ch)
                      for jt in range(NT_I) for ch in range(2)]
            if t + 1 < NPAIR:
                f0 += [lambda ch=ch: gen_qk_chunk(t + 1, 0, ch)
                       for ch in range(2)]
                f1 += [lambda ch=ch: gen_qk_chunk(t + 1, 1, ch)
                       for ch in range(2)]
            if t == 4:
                # partial projection over finished pairs (0-2) rides in
                # pair 4's attention
                f0 += [lambda it=it: proj_part(it, [0, 1, 2], True, False)
                       for it in range(0, 4)]
                f1 += [lambda it=it: proj_part(it, [0, 1, 2], True, False)
                       for it in range(4, NT_I)]
            if t == 5:
                # pairs 3-4 ride in pair 5's first half; the final k=5
                # chunk for row-tiles 0-3 (they only need the ib=0 half
                # of ot chunk 5) rides in the second half
                f0 += [lambda it=it: proj_part(it, [3, 4], False, False)
                       for it in range(NT_I)]
                f1 += [lambda it=it: proj_part(it, [5], False, True)
                       for it in range(0, 4)]
            attn_pair(t, 0, f0)
            attn_pair(t, 1, f1)

        # tail: last c_in chunk for the row-tiles needing pair 5's ib=1
        for it in range(4, NT_I):
            proj_part(it, [5], False, True)

    nc.compile()
    nc.finalize()
    return nc


_NC_CACHE = {}


def _get_nc(mode=None):
    if "nc" not in _NC_CACHE:
        _NC_CACHE["nc"] = build()
    return _NC_CACHE["nc"]


def _prep_host(x, w_qkv, w_proj, b_proj, mode=None):
    import ml_dtypes
    bf16 = ml_dtypes.bfloat16

    xt = np.ascontiguousarray(
        np.asarray(x).transpose(0, 2, 1)).astype(bf16)       # [B, C, N]
    wq_t = np.ascontiguousarray(w_qkv[0:C].T).astype(bf16)   # [C, C]
    wk_t = np.ascontiguousarray(w_qkv[C:2 * C].T).astype(bf16)
    wv_t = np.ascontiguousarray(w_qkv[2 * C:3 * C].T).astype(bf16)
    wp_t = np.ascontiguousarray(w_proj.T).astype(bf16)
    bias_rep = np.ascontiguousarray(
        np.broadcast_to(np.asarray(b_proj, dtype=np.float32), (128, C)))
    return xt, wq_t, wk_t, wv_t, wp_t, bias_rep


def run(x, w_qkv, w_proj, b_proj, mode=None, trace=False):
    nc = _get_nc()
    xt, wq_t, wk_t, wv_t, wp_t, bias_rep = _prep_host(x, w_qkv, w_proj, b_proj)
    in_maps = [
        {"xt": np.ascontiguousarray(xt[b]), "wq": wq_t, "wk": wk_t,
         "wv": wv_t, "wp": wp_t, "bias": bias_rep}
        for b in range(B)
    ]
    res = run_bass_kernel_spmd(
        nc, in_maps, core_ids=list(range(B)), trace=trace
    )
    out = np.stack([res.results[b]["y"] for b in range(B)]).astype(np.float32)
    return out, res


def kernel(x, w_qkv, w_proj, b_proj):
    out, _ = run(x, w_qkv, w_proj, b_proj)
    return out


# revision 29
# speedup vs baseline: 1.0583x; 1.0583x over previous
"""Multi-head attention (B=8, N=1024, C=768, H=12) on 8 TRN2 NeuronCores.

Sharding: data-parallel - one batch element per core, weights replicated.
No collectives.

v2 design (vs baseline): bf16 matmul operands everywhere (f32 PSUM), query
dim split in 512-halves so PSUM fits 8 banks with QKV interleave slots,
software-pipelined S->exp->PV per j-tile, QKV/V generation interleaved into
the attention loop as PE gap-filler (keeps HAM warm), reciprocal via the
fast custom-DVE approx, partition broadcast on GPSIMD instead of 7 chained
DMAs.

Per-core dataflow:
  qt/kt [128, pair, 1024]: rows = head-pair dims (A at 0:64, B at 64:128).
  v_sb [128 j, jt, head, 65]: col 64 is ones -> PV row 64 = softmax denom.
  Per (pair t, half ib, jtile): S^T halves via two K=64 matmuls row-packed
  into the PE halves, one exp ACTIVATE (FD=1024, scale fused), PV accumulates
  O^T[65, 512] over jt. Normalize: denom row -> reciprocal_approx_fast ->
  partition_broadcast -> multiply into ot_sb (bf16).
  Proj: y = OT.T @ wp + bias per 128-row tile, straight to DRAM.
"""

from contextlib import ExitStack

import numpy as np

import concourse.bacc as bacc
import concourse.mybir as mybir
import concourse.tile as tile
from concourse.bass_utils import run_bass_kernel_spmd

F32 = mybir.dt.float32
BF16 = mybir.dt.bfloat16

B, N, C = 8, 1024, 768
H, HD = 12, 64
SCALE = HD ** -0.5
NT_I = N // 128   # 8 i/j tiles
NT_C = C // 128   # 6 c tiles (== head pairs)
NPAIR = H // 2    # 6


def build():
    nc = bacc.Bacc(None, target_bir_lowering=False)

    xt = nc.dram_tensor("xt", [C, N], BF16, kind="ExternalInput")
    wq = nc.dram_tensor("wq", [C, C], BF16, kind="ExternalInput")
    wk = nc.dram_tensor("wk", [C, C], BF16, kind="ExternalInput")
    wv = nc.dram_tensor("wv", [C, C], BF16, kind="ExternalInput")
    wp = nc.dram_tensor("wp", [C, C], BF16, kind="ExternalInput")
    bias = nc.dram_tensor("bias", [128, C], F32, kind="ExternalInput")
    y = nc.dram_tensor("y", [N, C], F32, kind="ExternalOutput")

    with tile.TileContext(nc) as tc, ExitStack() as stack:
        pp = stack.enter_context(tc.tile_pool(name="persist", bufs=1))
        p_pt = stack.enter_context(tc.tile_pool(name="pt", bufs=6))
        p_nrm = stack.enter_context(tc.tile_pool(name="nrm", bufs=2))
        p_y = stack.enter_context(tc.tile_pool(name="yout", bufs=2))
        ps_qkv = stack.enter_context(
            tc.tile_pool(name="psq", bufs=2, space="PSUM"))
        ps_st = stack.enter_context(
            tc.tile_pool(name="psst", bufs=2, space="PSUM"))
        ps_ov = stack.enter_context(
            tc.tile_pool(name="psov", bufs=2, space="PSUM"))

        xt_sb = pp.tile([128, NT_C, N], BF16)
        wq_sb = pp.tile([128, NT_C, C], BF16)
        wk_sb = pp.tile([128, NT_C, C], BF16)
        wv_sb = pp.tile([128, NT_C, C], BF16)
        wp_sb = pp.tile([128, NT_C, C], BF16)
        bias_sb = pp.tile([128, C], F32)
        qt_sb = pp.tile([128, NPAIR, N], BF16)
        kt_sb = pp.tile([128, NPAIR, N], BF16)
        v_sb = pp.tile([128, NT_I, H, HD + 1], BF16)
        ot_sb = pp.tile([128, NPAIR, N], BF16)

        # Input DMAs: big contiguous transfers on the Sync queue in
        # consumption order (spreading across Scalar/GpSimd injects DMA
        # sem-waits into queues carrying exp / partition_broadcast and
        # measured slower; fine-grained chunking measured no better).
        nc.sync.dma_start(xt_sb[:], xt.rearrange("(t p) i -> p t i", p=128))
        nc.sync.dma_start(wq_sb[:], wq.rearrange("(t p) d -> p t d", p=128))
        nc.sync.dma_start(wk_sb[:], wk.rearrange("(t p) d -> p t d", p=128))
        nc.sync.dma_start(wv_sb[:], wv.rearrange("(t p) d -> p t d", p=128))
        nc.sync.dma_start(wp_sb[:], wp.rearrange("(t p) d -> p t d", p=128))
        nc.sync.dma_start(bias_sb[:], bias[:])
        nc.vector.memset(v_sb[:, :, :, HD:HD + 1], 1.0)

        def gen_qk_chunk(t, which, ch):
            """One accumulation chain of Q.T (which=0) or K.T (which=1)."""
            w_sb, out_sb = ((wq_sb, qt_sb), (wk_sb, kt_sb))[which]
            acc = ps_qkv.tile([128, 512], F32, tag="acc",
                              name=f"qk{t}_{which}_{ch}")
            for k in range(NT_C):
                nc.tensor.matmul(
                    acc[:],
                    w_sb[:, k, t * 128:(t + 1) * 128],
                    xt_sb[:, k, ch * 512:(ch + 1) * 512],
                    start=(k == 0), stop=(k == NT_C - 1),
                )
            nc.vector.tensor_copy(out_sb[:, t, ch * 512:(ch + 1) * 512],
                                  acc[:])

        def gen_v_chunk(jt, ch):
            acc = ps_qkv.tile([128, 384], F32, tag="acc",
                              name=f"v{jt}_{ch}")
            for k in range(NT_C):
                nc.tensor.matmul(
                    acc[:],
                    xt_sb[:, k, jt * 128:(jt + 1) * 128],
                    wv_sb[:, k, ch * 384:(ch + 1) * 384],
                    start=(k == 0), stop=(k == NT_C - 1),
                )
            nc.vector.tensor_copy(
                v_sb[:, jt, 6 * ch:6 * ch + 6, 0:HD],
                acc[:].rearrange("p (h e) -> p h e", e=HD),
            )

        def attn_pair(t, ib, filler):
            """Attention for head pair t on query half ib (512 queries).

            filler: list of zero-arg emitters (extra PE work) drained a few
            per jt step so the scheduler has gap-fill matmuls while ACT
            runs exp. Emitted at the BOTTOM of each step so they rank below
            the attention instructions in scheduler priority (pure
            gap-fill). Drained fast enough that all run by step NT_I-2.
            """
            i0 = ib * 512
            hA, hB = 2 * t, 2 * t + 1
            per_step = -(-len(filler) // (NT_I - 1)) if filler else 0
            ovA = ps_ov.tile([HD + 1, 512], F32, tag="ov",
                             name=f"ovA{t}_{ib}")
            ovB = ps_ov.tile([HD + 1, 512], F32, tag="ov",
                             name=f"ovB{t}_{ib}")
            pts = [None] * NT_I
            for jt in range(NT_I + 1):
                if jt < NT_I:
                    st = ps_st.tile([128, 1024], F32, tag="st",
                                    name=f"st{t}_{ib}_{jt}")
                    nc.tensor.matmul(
                        st[:, 0:512],
                        kt_sb[0:64, t, jt * 128:(jt + 1) * 128],
                        qt_sb[0:64, t, i0:i0 + 512],
                    )
                    nc.tensor.matmul(
                        st[:, 512:1024],
                        kt_sb[64:128, t, jt * 128:(jt + 1) * 128],
                        qt_sb[64:128, t, i0:i0 + 512],
                    )
                    pt = p_pt.tile([128, 1024], BF16, tag="pt")
                    nc.scalar.activation(
                        pt[:], st[:],
                        mybir.ActivationFunctionType.Exp, scale=SCALE,
                    )
                    pts[jt] = pt
                if jt > 0:
                    j = jt - 1
                    pt = pts[j]
                    nc.tensor.matmul(
                        ovA[:], v_sb[:, j, hA, :], pt[:, 0:512],
                        start=(j == 0), stop=(j == NT_I - 1),
                    )
                    nc.tensor.matmul(
                        ovB[:], v_sb[:, j, hB, :], pt[:, 512:1024],
                        start=(j == 0), stop=(j == NT_I - 1),
                    )
                for _ in range(per_step):
                    if filler:
                        filler.pop(0)()
            # normalize: denom row 64 -> 1/l -> broadcast -> multiply
            for base, ov in ((0, ovA), (64, ovB)):
                rl = p_nrm.tile([1, 512], F32, tag="rl")
                rc = p_nrm.tile([1, 512], F32, tag="rc")
                bc = p_nrm.tile([128, 512], F32, tag="bc")
                nc.vector.tensor_copy(rl[0:1, :], ov[64:65, :])
                nc.vector.reciprocal_approx_fast(rc[0:1, :], rl[0:1, :])
                nc.gpsimd.partition_broadcast(bc[:], rc[0:1, :])
                nc.vector.tensor_mul(
                    ot_sb[base:base + 64, t, i0:i0 + 512],
                    ov[0:64, :],
                    bc[base:base + 64, :],
                )

        def proj(it):
            y_sb = p_y.tile([128, C], F32, tag="y", name=f"y{it}")
            for ch in range(2):
                acc = ps_qkv.tile([128, 384], F32, tag="acc",
                                  name=f"p{it}_{ch}")
                for k in range(NT_C):
                    nc.tensor.matmul(
                        acc[:],
                        ot_sb[:, k, it * 128:(it + 1) * 128],
                        wp_sb[:, k, ch * 384:(ch + 1) * 384],
                        start=(k == 0), stop=(k == NT_C - 1),
                    )
                sl = slice(ch * 384, (ch + 1) * 384)
                nc.vector.tensor_add(y_sb[:, sl], acc[:], bias_sb[:, sl])
            nc.sync.dma_start(y[it * 128:(it + 1) * 128, :], y_sb[:])

        # prologue: Q.T/K.T chains for pair 0 (V rides in the fillers so
        # a late wv DMA cannot block the first S matmuls in PE order)
        for ch in range(2):
            gen_qk_chunk(0, 0, ch)
            gen_qk_chunk(0, 1, ch)

        for t in range(NPAIR):
            # fillers: extra PE chains emitted at the bottom of the
            # attention jt steps so the scheduler has gap-fill matmuls
            # while ACT runs exp.
            f0, f1 = [], []
            if t == 0:
                # V tiles inside pair 0; V[j] lands before the PV reads it
                f0 = [lambda jt=jt, ch=ch: gen_v_chunk(jt, ch)
                      for jt in range(NT_I) for ch in range(2)]
            if t + 1 < NPAIR:
                f0 += [lambda ch=ch: gen_qk_chunk(t + 1, 0, ch)
                       for ch in range(2)]
                f1 += [lambda ch=ch: gen_qk_chunk(t + 1, 1, ch)
                       for ch in range(2)]
            if t == 5:
                # projection of row-tiles 0-3 reads only the ib=0 columns
                # of every ot chunk, so the full chains ride in pair 5's
                # second half, right after norm(5, ib0)
                f1 += [lambda it=it: proj(it) for it in range(0, 4)]
            attn_pair(t, 0, f0)
            attn_pair(t, 1, f1)

        # tail: row-tiles that need pair 5's ib=1 half
        for it in range(4, NT_I):
            proj(it)

    nc.compile()
    nc.finalize()
    return nc


_NC_CACHE = {}


def _get_nc(mode=None):
    if "nc" not in _NC_CACHE:
        _NC_CACHE["nc"] = build()
    return _NC_CACHE["nc"]


def _prep_host(x, w_qkv, w_proj, b_proj, mode=None):
    import ml_dtypes
    bf16 = ml_dtypes.bfloat16

    xt = np.ascontiguousarray(
        np.asarray(x).transpose(0, 2, 1)).astype(bf16)       # [B, C, N]
    wq_t = np.ascontiguousarray(w_qkv[0:C].T).astype(bf16)   # [C, C]
    wk_t = np.ascontiguousarray(w_qkv[C:2 * C].T).astype(bf16)
    wv_t = np.ascontiguousarray(w_qkv[2 * C:3 * C].T).astype(bf16)
    wp_t = np.ascontiguousarray(w_proj.T).astype(bf16)
    bias_rep = np.ascontiguousarray(
        np.broadcast_to(np.asarray(b_proj, dtype=np.float32), (128, C)))
    return xt, wq_t, wk_t, wv_t, wp_t, bias_rep


def run(x, w_qkv, w_proj, b_proj, mode=None, trace=False):
    nc = _get_nc()
    xt, wq_t, wk_t, wv_t, wp_t, bias_rep = _prep_host(x, w_qkv, w_proj, b_proj)
    in_maps = [
        {"xt": np.ascontiguousarray(xt[b]), "wq": wq_t, "wk": wk_t,
         "wv": wv_t, "wp": wp_t, "bias": bias_rep}
        for b in range(B)
    ]
    res = run_bass_kernel_spmd(
        nc, in_maps, core_ids=list(range(B)), trace=trace
    )
    out = np.stack([res.results[b]["y"] for b in range(B)]).astype(np.float32)
    return out, res


def kernel(x, w_qkv, w_proj, b_proj):
    out, _ = run(x, w_qkv, w_proj, b_proj)
    return out


# revision 31
# speedup vs baseline: 1.0707x; 1.0117x over previous
"""Multi-head attention (B=8, N=1024, C=768, H=12) on 8 TRN2 NeuronCores.

Sharding: data-parallel - one batch element per core, weights replicated.
No collectives.

v2 design (vs baseline): bf16 matmul operands everywhere (f32 PSUM), query
dim split in 512-halves so PSUM fits 8 banks with QKV interleave slots,
software-pipelined S->exp->PV per j-tile, QKV/V generation interleaved into
the attention loop as PE gap-filler (keeps HAM warm), reciprocal via the
fast custom-DVE approx, partition broadcast on GPSIMD instead of 7 chained
DMAs.

Per-core dataflow:
  qt/kt [128, pair, 1024]: rows = head-pair dims (A at 0:64, B at 64:128).
  v_sb [128 j, jt, head, 65]: col 64 is ones -> PV row 64 = softmax denom.
  Per (pair t, half ib, jtile): S^T halves via two K=64 matmuls row-packed
  into the PE halves, one exp ACTIVATE (FD=1024, scale fused), PV accumulates
  O^T[65, 512] over jt. Normalize: denom row -> reciprocal_approx_fast ->
  partition_broadcast -> multiply into ot_sb (bf16).
  Proj: y = OT.T @ wp + bias per 128-row tile, straight to DRAM.
"""

from contextlib import ExitStack

import numpy as np

import concourse.bacc as bacc
import concourse.mybir as mybir
import concourse.tile as tile
from concourse.bass_utils import run_bass_kernel_spmd

F32 = mybir.dt.float32
BF16 = mybir.dt.bfloat16

B, N, C = 8, 1024, 768
H, HD = 12, 64
SCALE = HD ** -0.5
NT_I = N // 128   # 8 i/j tiles
NT_C = C // 128   # 6 c tiles (== head pairs)
NPAIR = H // 2    # 6


def build():
    nc = bacc.Bacc(None, target_bir_lowering=False)

    xt = nc.dram_tensor("xt", [C, N], BF16, kind="ExternalInput")
    wq = nc.dram_tensor("wq", [C, C], BF16, kind="ExternalInput")
    wk = nc.dram_tensor("wk", [C, C], BF16, kind="ExternalInput")
    wv = nc.dram_tensor("wv", [C, C], BF16, kind="ExternalInput")
    wp = nc.dram_tensor("wp", [C, C], BF16, kind="ExternalInput")
    bias = nc.dram_tensor("bias", [128, C], F32, kind="ExternalInput")
    y = nc.dram_tensor("y", [N, C], F32, kind="ExternalOutput")

    with tile.TileContext(nc) as tc, ExitStack() as stack:
        pp = stack.enter_context(tc.tile_pool(name="persist", bufs=1))
        p_pt = stack.enter_context(tc.tile_pool(name="pt", bufs=6))
        p_nrm = stack.enter_context(tc.tile_pool(name="nrm", bufs=2))
        p_y = stack.enter_context(tc.tile_pool(name="yout", bufs=2))
        ps_qkv = stack.enter_context(
            tc.tile_pool(name="psq", bufs=2, space="PSUM"))
        ps_st = stack.enter_context(
            tc.tile_pool(name="psst", bufs=2, space="PSUM"))
        ps_ov = stack.enter_context(
            tc.tile_pool(name="psov", bufs=2, space="PSUM"))

        xt_sb = pp.tile([128, NT_C, N], BF16)
        wq_sb = pp.tile([128, NT_C, C], BF16)
        wk_sb = pp.tile([128, NT_C, C], BF16)
        wv_sb = pp.tile([128, NT_C, C], BF16)
        wp_sb = pp.tile([128, NT_C, C], BF16)
        bias_sb = pp.tile([128, C], F32)
        qt_sb = pp.tile([128, NPAIR, N], BF16)
        kt_sb = pp.tile([128, NPAIR, N], BF16)
        v_sb = pp.tile([128, NT_I, H, HD + 1], BF16)
        ot_sb = pp.tile([128, NPAIR, N], BF16)

        # Input DMAs: all on the Sync queue (issue on Scalar/GpSimd
        # injects DMA sem-waits into queues carrying exp /
        # partition_broadcast and measured slower). Each tensor is split
        # in two so the transfers fan out across more HW DGE queues
        # (aggregate ~330GB/s vs ~220 for monolithic), in consumption
        # order.
        def dma2(dst, src):
            h = NT_C // 2
            nc.sync.dma_start(
                dst[:, 0:h], src[0:h * 128].rearrange("(t p) i -> p t i",
                                                      p=128))
            nc.sync.dma_start(
                dst[:, h:NT_C],
                src[h * 128:NT_C * 128].rearrange("(t p) i -> p t i", p=128))

        dma2(xt_sb, xt)
        dma2(wq_sb, wq)
        dma2(wk_sb, wk)
        dma2(wv_sb, wv)
        dma2(wp_sb, wp)
        nc.sync.dma_start(bias_sb[:], bias[:])
        nc.vector.memset(v_sb[:, :, :, HD:HD + 1], 1.0)

        def gen_qk_chunk(t, which, ch):
            """One accumulation chain of Q.T (which=0) or K.T (which=1)."""
            w_sb, out_sb = ((wq_sb, qt_sb), (wk_sb, kt_sb))[which]
            acc = ps_qkv.tile([128, 512], F32, tag="acc",
                              name=f"qk{t}_{which}_{ch}")
            for k in range(NT_C):
                nc.tensor.matmul(
                    acc[:],
                    w_sb[:, k, t * 128:(t + 1) * 128],
                    xt_sb[:, k, ch * 512:(ch + 1) * 512],
                    start=(k == 0), stop=(k == NT_C - 1),
                )
            nc.vector.tensor_copy(out_sb[:, t, ch * 512:(ch + 1) * 512],
                                  acc[:])

        def gen_v_chunk(jt, ch):
            acc = ps_qkv.tile([128, 384], F32, tag="acc",
                              name=f"v{jt}_{ch}")
            for k in range(NT_C):
                nc.tensor.matmul(
                    acc[:],
                    xt_sb[:, k, jt * 128:(jt + 1) * 128],
                    wv_sb[:, k, ch * 384:(ch + 1) * 384],
                    start=(k == 0), stop=(k == NT_C - 1),
                )
            nc.vector.tensor_copy(
                v_sb[:, jt, 6 * ch:6 * ch + 6, 0:HD],
                acc[:].rearrange("p (h e) -> p h e", e=HD),
            )

        def attn_pair(t, ib, filler):
            """Attention for head pair t on query half ib (512 queries).

            filler: list of zero-arg emitters (extra PE work) drained a few
            per jt step so the scheduler has gap-fill matmuls while ACT
            runs exp. Emitted at the BOTTOM of each step so they rank below
            the attention instructions in scheduler priority (pure
            gap-fill). Drained fast enough that all run by step NT_I-2.
            """
            i0 = ib * 512
            hA, hB = 2 * t, 2 * t + 1
            per_step = -(-len(filler) // (NT_I - 1)) if filler else 0
            ovA = ps_ov.tile([HD + 1, 512], F32, tag="ov",
                             name=f"ovA{t}_{ib}")
            ovB = ps_ov.tile([HD + 1, 512], F32, tag="ov",
                             name=f"ovB{t}_{ib}")
            pts = [None] * NT_I
            for jt in range(NT_I + 1):
                if jt < NT_I:
                    st = ps_st.tile([128, 1024], F32, tag="st",
                                    name=f"st{t}_{ib}_{jt}")
                    nc.tensor.matmul(
                        st[:, 0:512],
                        kt_sb[0:64, t, jt * 128:(jt + 1) * 128],
                        qt_sb[0:64, t, i0:i0 + 512],
                    )
                    nc.tensor.matmul(
                        st[:, 512:1024],
                        kt_sb[64:128, t, jt * 128:(jt + 1) * 128],
                        qt_sb[64:128, t, i0:i0 + 512],
                    )
                    pt = p_pt.tile([128, 1024], BF16, tag="pt")
                    nc.scalar.activation(
                        pt[:], st[:],
                        mybir.ActivationFunctionType.Exp, scale=SCALE,
                    )
                    pts[jt] = pt
                if jt > 0:
                    j = jt - 1
                    pt = pts[j]
                    nc.tensor.matmul(
                        ovA[:], v_sb[:, j, hA, :], pt[:, 0:512],
                        start=(j == 0), stop=(j == NT_I - 1),
                    )
                    nc.tensor.matmul(
                        ovB[:], v_sb[:, j, hB, :], pt[:, 512:1024],
                        start=(j == 0), stop=(j == NT_I - 1),
                    )
                for _ in range(per_step):
                    if filler:
                        filler.pop(0)()
            # normalize: denom row 64 -> 1/l -> broadcast -> multiply
            for base, ov in ((0, ovA), (64, ovB)):
                rl = p_nrm.tile([1, 512], F32, tag="rl")
                rc = p_nrm.tile([1, 512], F32, tag="rc")
                bc = p_nrm.tile([128, 512], F32, tag="bc")
                nc.vector.tensor_copy(rl[0:1, :], ov[64:65, :])
                nc.vector.reciprocal_approx_fast(rc[0:1, :], rl[0:1, :])
                nc.gpsimd.partition_broadcast(bc[:], rc[0:1, :])
                nc.vector.tensor_mul(
                    ot_sb[base:base + 64, t, i0:i0 + 512],
                    ov[0:64, :],
                    bc[base:base + 64, :],
                )

        def proj(it):
            y_sb = p_y.tile([128, C], F32, tag="y", name=f"y{it}")
            for ch in range(2):
                acc = ps_qkv.tile([128, 384], F32, tag="acc",
                                  name=f"p{it}_{ch}")
                for k in range(NT_C):
                    nc.tensor.matmul(
                        acc[:],
                        ot_sb[:, k, it * 128:(it + 1) * 128],
                        wp_sb[:, k, ch * 384:(ch + 1) * 384],
                        start=(k == 0), stop=(k == NT_C - 1),
                    )
                sl = slice(ch * 384, (ch + 1) * 384)
                nc.vector.tensor_add(y_sb[:, sl], acc[:], bias_sb[:, sl])
            nc.sync.dma_start(y[it * 128:(it + 1) * 128, :], y_sb[:])

        # prologue: Q.T/K.T chains for pair 0 (V rides in the fillers so
        # a late wv DMA cannot block the first S matmuls in PE order)
        for ch in range(2):
            gen_qk_chunk(0, 0, ch)
            gen_qk_chunk(0, 1, ch)

        for t in range(NPAIR):
            # fillers: extra PE chains emitted at the bottom of the
            # attention jt steps so the scheduler has gap-fill matmuls
            # while ACT runs exp.
            f0, f1 = [], []
            if t == 0:
                # V tiles inside pair 0; V[j] lands before the PV reads it
                f0 = [lambda jt=jt, ch=ch: gen_v_chunk(jt, ch)
                      for jt in range(NT_I) for ch in range(2)]
            if t + 1 < NPAIR:
                f0 += [lambda ch=ch: gen_qk_chunk(t + 1, 0, ch)
                       for ch in range(2)]
                f1 += [lambda ch=ch: gen_qk_chunk(t + 1, 1, ch)
                       for ch in range(2)]
            if t == 5:
                # projection of row-tiles 0-3 reads only the ib=0 columns
                # of every ot chunk, so the full chains ride in pair 5's
                # second half, right after norm(5, ib0)
                f1 += [lambda it=it: proj(it) for it in range(0, 4)]
            attn_pair(t, 0, f0)
            attn_pair(t, 1, f1)

        # tail: row-tiles that need pair 5's ib=1 half
        for it in range(4, NT_I):
            proj(it)

    nc.compile()
    nc.finalize()
    return nc


_NC_CACHE = {}


def _get_nc(mode=None):
    if "nc" not in _NC_CACHE:
        _NC_CACHE["nc"] = build()
    return _NC_CACHE["nc"]


def _prep_host(x, w_qkv, w_proj, b_proj, mode=None):
    import ml_dtypes
    bf16 = ml_dtypes.bfloat16

    xt = np.ascontiguousarray(
        np.asarray(x).transpose(0, 2, 1)).astype(bf16)       # [B, C, N]
    wq_t = np.ascontiguousarray(w_qkv[0:C].T).astype(bf16)   # [C, C]
    wk_t = np.ascontiguousarray(w_qkv[C:2 * C].T).astype(bf16)
    wv_t = np.ascontiguousarray(w_qkv[2 * C:3 * C].T).astype(bf16)
    wp_t = np.ascontiguousarray(w_proj.T).astype(bf16)
    bias_rep = np.ascontiguousarray(
        np.broadcast_to(np.asarray(b_proj, dtype=np.float32), (128, C)))
    return xt, wq_t, wk_t, wv_t, wp_t, bias_rep


def run(x, w_qkv, w_proj, b_proj, mode=None, trace=False):
    nc = _get_nc()
    xt, wq_t, wk_t, wv_t, wp_t, bias_rep = _prep_host(x, w_qkv, w_proj, b_proj)
    in_maps = [
        {"xt": np.ascontiguousarray(xt[b]), "wq": wq_t, "wk": wk_t,
         "wv": wv_t, "wp": wp_t, "bias": bias_rep}
        for b in range(B)
    ]
    res = run_bass_kernel_spmd(
        nc, in_maps, core_ids=list(range(B)), trace=trace
    )
    out = np.stack([res.results[b]["y"] for b in range(B)]).astype(np.float32)
    return out, res


def kernel(x, w_qkv, w_proj, b_proj):
    out, _ = run(x, w_qkv, w_proj, b_proj)
    return out
